# revision 2
# baseline (speedup 1.0000x reference)
"""BitLinear-1.58 inference kernel for Trainium2 (8 NeuronCores, token-parallel).

out = (clip(round(x * 128/gamma), -128, 127) @ W^T) * (scale*gamma/128) + bias
with gamma = max(|x|, axis=-1), W ternary {-1, 0, 1}.

PE runs fp8e4m3 matmuls with perf_mode=DoubleRow (2 fp8 MACs/cell/cycle,
K=256 per matmul). xq in [-128,127] is not exact in fp8e4m3, so channels
d<1024 use the exact split hi = f8(xq), lo = f8(xq - hi) (slots share one
weight copy via the same rhs AP), channels d>=1024 use plain f8(xq).
The fp8 data is packed pre-transpose as byte pairs ((hi,lo) interleaved for
the exact half, natural adjacent-d for the plain half), bitcast to f16 and
transposed by the XBAR in one pass; DoubleRow stationary APs then read the
byte-interleaved slots directly (pair stride 256B, column stride 2B).
Quantization uses the f16 magic-round trick (x*inv + 1536, RNE). x is
host-cast to f16 and out is stored as f16 (lossless host upcast).
"""

import os
import numpy as np
import ml_dtypes
from contextlib import ExitStack


def _env(k, d):
    return int(os.environ.get(k, d))


LABEL_HOOK = None


def _lbl(x):
    if LABEL_HOOK is not None:
        LABEL_HOOK["label"] = x

import concourse.bass as bass
import concourse.mybir as mybir
import concourse.tile as tile
from concourse import bacc
from concourse.bass_utils import run_bass_kernel_spmd

N_CORES = 8
B, S, D_IN, D_OUT = 4, 4096, 2048, 2048
TOKENS = B * S                 # 16384
TPC = TOKENS // N_CORES        # 2048 tokens per core
P = 128
N_TILES = TPC // P             # 16 token tiles per core
ED = D_IN // 2                 # 1024 exact channels (hi/lo), rest plain f8
NEB = 2 * ED // 256            # 8 exact c'-blocks (256B each)
NPB = (D_IN - ED) // 256       # 4 plain c'-blocks
NB = NEB + NPB + 2             # 12 data c'-blocks + bias slot + pad
PKB = 256 * NB                 # 3584 packed bytes per token
NF = 512                       # matmul free dim (one PSUM bank of fp32)
OC = D_OUT // NF               # 4 output chunks
MAGIC = 1536.0                 # 1.5 * 2**10: f16 round-half-even shift
EPS = 1e-5
Q = 128.0

F32 = mybir.dt.float32
F16 = mybir.dt.float16
F8 = mybir.dt.float8e4
AX = mybir.AxisListType
OP = mybir.AluOpType
AF = mybir.ActivationFunctionType
PM = mybir.MatmulPerfMode


def build_kernel(n_tiles=N_TILES):
    nc = bacc.Bacc(
        "TRN2", target_bir_lowering=False, debug=False, num_devices=N_CORES
    )
    tpc = n_tiles * P
    x_d = nc.dram_tensor("x", [tpc, D_IN], F16, kind="ExternalInput").ap()
    # we: [p, cc(8), o] = W^T[128cc + p, o]   (exact half, shared by hi and lo)
    # wp: [p, j(2), b(2), s(2), o] = W^T[1024 + 512j + 256b + 2p + s, o]
    w_d = nc.dram_tensor("w", [P, (NEB + NPB * 2) * D_OUT], F8,
                         kind="ExternalInput").ap()
    wbr_d = nc.dram_tensor("wbias", [1, D_OUT], F8, kind="ExternalInput").ap()
    b_d = nc.dram_tensor("rs16", [P, 1], F32, kind="ExternalInput").ap()
    s_d = nc.dram_tensor("scale", [P, 1], F32, kind="ExternalInput").ap()
    o_d = nc.dram_tensor("out", [tpc, D_OUT], F16, kind="ExternalOutput").ap()

    with tile.TileContext(nc) as tc:
        with ExitStack() as ctx:
            _emit(ctx, tc, o_d, x_d, w_d, b_d, s_d, n_tiles, wbr_d)
    _dedup_ldweights(nc)
    nc.compile()
    return nc


def _dedup_ldweights(nc):
    """Drop InstLdweights whose weights AP matches the previous LDW in the
    same block (PE stationary registers still hold those weights)."""
    n_removed = 0
    for bb in nc.main_func.blocks:
        kept = []
        last_key = None
        pending_waits = []
        for inst in bb.instructions:
            if isinstance(inst, mybir.InstLdweights):
                key = repr(inst.ins)
                if key == last_key:
                    si = inst.sync_info
                    if si is not None and si.on_wait:
                        pending_waits.extend(si.on_wait)
                    n_removed += 1
                    continue
                last_key = key
            elif isinstance(inst, (mybir.InstMatmult, mybir.InstEventSemaphore)):
                pass  # does not clobber PE stationary weights
            elif getattr(inst, "engine", None) == mybir.EngineType.PE:
                last_key = None  # conservative: other PE instruction
            if pending_waits and getattr(inst, "engine", None) == mybir.EngineType.PE:
                si = inst.sync_info
                if si is None:
                    inst.sync_info = mybir.SyncInfo(
                        on_wait=list(pending_waits), on_update=[]
                    )
                else:
                    si.on_wait = list(si.on_wait) + pending_waits
                pending_waits = []
            kept.append(inst)
        assert not pending_waits, "dangling waits from dropped trailing LDW"
        bb.instructions[:] = kept
    return n_removed


def _emit(ctx, tc, o_d, x_d, w_d, b_d, s_d, n_tiles, wbr_d=None):
    nc = tc.nc

    const = ctx.enter_context(tc.tile_pool(name="const", bufs=1))
    xp = ctx.enter_context(tc.tile_pool(name="xp", bufs=_env("K_XP", 8)))
    tp = ctx.enter_context(tc.tile_pool(name="tp", bufs=_env("K_TP", 5)))
    qp = ctx.enter_context(tc.tile_pool(name="qp", bufs=_env("K_QP", 5)))
    pkp = ctx.enter_context(tc.tile_pool(name="pkp", bufs=_env("K_PKP", 5)))
    ptp = ctx.enter_context(tc.tile_pool(name="ptp", bufs=_env("K_PTP", 5)))
    outp = ctx.enter_context(tc.tile_pool(name="outp", bufs=_env("K_OUTP", 3)))
    smp = ctx.enter_context(tc.tile_pool(name="smp", bufs=_env("K_SMP", 7)))
    psp = ctx.enter_context(tc.tile_pool(name="psp", bufs=_env("K_PSB", 2), space="PSUM"))

    XA = _env("K_XA", 4)
    x_tiles = {}

    def fetch_x(i):
        if i < n_tiles and i not in x_tiles:
            x_t = xp.tile([P, D_IN], F16, tag="x")
            nc.sync.dma_start(x_t[:], x_d[i * P : (i + 1) * P, :])
            x_tiles[i] = x_t

    for i in range(XA):
        fetch_x(i)

    magic_sb = const.tile([P, 1], F32)
    nc.any.memset(magic_sb[:], MAGIC)
    nmagic_sb = const.tile([P, 1], F32)
    nc.any.memset(nmagic_sb[:], -MAGIC)
    # touch ScalarE once so its activation table load runs during startup fill
    warm_act = const.tile([P, 1], F32)
    nc.scalar.activation(warm_act[:], magic_sb[:], AF.Identity, bias=magic_sb[:, 0:1])
    scale_sb = const.tile([P, 1], F32)
    nc.sync.dma_start(scale_sb[:], s_d[:])
    rs16_sb = const.tile([P, 1], F32)
    nc.sync.dma_start(rs16_sb[:], b_d[:])

    w_sb = const.tile([P, (NEB + NPB * 2 + 2) * D_OUT], F8)
    # exact-half weights: [p, cc(8), o]
    we3 = w_sb[:, : NEB * D_OUT].rearrange("p (cc o) -> p cc o", cc=NEB)
    # plain-half weights: [p, j(2), b(2), s(2), o]
    wp5 = w_sb[:, NEB * D_OUT : (NEB + 2 * NPB) * D_OUT].rearrange(
        "p (j b s o) -> p j b s o", j=NPB // 2, b=2, s=2
    )
    # bias weights: [p, b(2), o]; row p=0,b=0 = 16*bias, rest zeros
    wb3 = w_sb[:, (NEB + 2 * NPB) * D_OUT :].rearrange("p (b o) -> p b o", b=2)
    nc.gpsimd.memset(w_sb[:, (NEB + 2 * NPB) * D_OUT :], 0)
    nc.sync.dma_start(wb3[0:1, 0, :], wbr_d[:])
    NWCH = NEB + NPB * 2  # 16 loaded weight chunks of D_OUT

    def load_w_chunk(g):
        nc.sync.dma_start(
            w_sb[:, g * D_OUT : (g + 1) * D_OUT],
            w_d[:, g * D_OUT : (g + 1) * D_OUT],
        )

    for g in range(_env("K_PRE", 4)):
        load_w_chunk(g)

    def quant_stage(i):
        """x -> gamma -> t1 -> {hi8, xq, pl8, lo8} -> packed transpose."""
        _lbl(f"QS{i}")
        fetch_x(i + XA)
        x_t = x_tiles.pop(i)

        import contextlib
        prio = tc.high_priority() if i < _env("K_HIPRI", 3) else contextlib.nullcontext()
        with prio:
            gamma = smp.tile([P, 1], F32, tag="gamma")
            nc.vector.tensor_reduce(
                gamma[:], x_t[:], axis=AX.X, op=OP.max, apply_absolute_value=True
            )
            g2 = smp.tile([P, 1], F32, tag="g2")
            nc.vector.tensor_scalar(g2[:], gamma[:], EPS, 1.0 / Q, OP.max, OP.mult)
            inv = smp.tile([P, 1], F32, tag="inv")
            nc.vector.reciprocal(inv[:], g2[:])
            deq = smp.tile([P, 1], F32, tag="deq")
            nc.vector.tensor_scalar(deq[:], g2[:], scale_sb[:, 0:1], None, OP.mult)

        _lbl(f"QS{i}.t1")
        # t1 = f16(x*inv + 1536): round-half-even to integer via f16 RNE.
        # DVE tensor_scalar (mult by inv ptr, add imm) runs in 4x 16-bit mode.
        t1 = tp.tile([P, D_IN], F16, tag="t1")
        nc.vector.tensor_scalar(t1[:], x_t[:], inv[:, 0:1], MAGIC, OP.mult, OP.add)

        pk = pkp.tile([P, PKB], F8, tag="pk")
        pkv = pk[:, : 2 * ED].rearrange("p (d two) -> p d two", two=2)
        _lbl(f"QS{i}.hi8")
        # hi8 = f8(t1 - 1536) straight off ScalarE (128 saturates to 128; the
        # lo residual corrects it against the clipped xq)
        nc.scalar.activation(
            pkv[:, :, 0], t1[:, :ED], AF.Identity, bias=nmagic_sb[:, 0:1]
        )
        _lbl(f"QS{i}.quant")
        # xq = f16(min(t1-1536, 127)) for the exact half only (DVE 4x mode)
        xq = qp.tile([P, ED], F16, tag="xq")
        nc.vector.tensor_scalar(xq[:], t1[:, :ED], MAGIC, Q - 1.0, OP.subtract, OP.min)
        _lbl(f"QS{i}.pl8")
        # plain half: pl8 = f8(min(t1-1536, 127)) on Pool
        nc.gpsimd.tensor_scalar(
            pk[:, 2 * ED : 2 * ED + (D_IN - ED)],
            t1[:, ED:], MAGIC, Q - 1.0, OP.subtract, OP.min
        )
        _lbl(f"QS{i}.lo8")
        # lo8 = xq - hi8 (Pool, stride-2 write)
        nc.gpsimd.tensor_tensor(pkv[:, :, 1], xq[:], pkv[:, :, 0], op=OP.subtract)

        _lbl(f"QS{i}.bias")
        # bias slot: byte 0 of block 12 at partition-row position; pads are
        # zeroed once per pool buffer (values persist across reuse)
        if i < _env("K_PKP", 5):
            nc.gpsimd.memset(pk[:, 2 * ED + (D_IN - ED) + 1 :], 0)
        nc.vector.tensor_scalar(
            pk[:, 2 * ED + (D_IN - ED) : 2 * ED + (D_IN - ED) + 1],
            inv[:],
            rs16_sb[:, 0:1],
            240.0,
            OP.mult,
            OP.min,
        )

        # remaining weight chunks spread over stages 0-2 (before mm_stage(0))
        wpre = _env("K_PRE", 4)
        wsp = _env("K_WSPREAD", 4)
        if i < 3:
            for g in range(wpre + i * wsp, min(wpre + (i + 1) * wsp, NWCH)):
                load_w_chunk(g)

        return pk, deq

    def transpose_stage(i, st):
        pk, deq = st
        _lbl(f"QS{i}.T")
        # packed XBAR transpose (16-bit view): [p, c', t]
        pkT = ptp.tile([P, PKB], F8, tag="pkT")
        pk16 = pk.bitcast(F16)
        pkT16 = pkT.bitcast(F16).rearrange("p (c t) -> p c t", c=NB)
        nsp = _env("K_TSPLIT", 2)
        hb = NB // nsp
        for sp in range(nsp):
            nc.sync.dma_start_transpose(
                pkT16[:, sp * hb : (sp + 1) * hb, :],
                pk16[:, sp * hb * P : (sp + 1) * hb * P],
            )

        return pkT, deq

    def mm_stage(i, st):
        _lbl(f"MM{i}")
        pkT, deq = st
        pkT4 = pkT.rearrange("p (c t two) -> p c t two", c=NB, two=2)
        # stationaries: exact block-pairs x {hi, lo}, then plain pairs x {s}
        stats = []
        for c in range(0, NEB, 2):
            stats.append((pkT4[:, c : c + 2, :, 0], we3[:, c : c + 2, :]))
            stats.append((pkT4[:, c : c + 2, :, 1], we3[:, c : c + 2, :]))
        for j in range(NPB // 2):
            cb = NEB + 2 * j
            stats.append((pkT4[:, cb : cb + 2, :, 0], wp5[:, j, :, 0, :]))
            stats.append((pkT4[:, cb : cb + 2, :, 1], wp5[:, j, :, 1, :]))
        cb = NEB + NPB
        stats.append((pkT4[:, cb : cb + 2, :, 0], wb3[:]))

        ps = psp.tile([P, D_OUT], F32, tag="ps")
        n_st = len(stats)
        for si_, (lhsT, rhsg) in enumerate(stats):
            for oc in range(OC):
                nc.tensor.matmul(
                    ps[:, oc * NF : (oc + 1) * NF],
                    lhsT,
                    rhsg[:, :, oc * NF : (oc + 1) * NF],
                    start=(si_ == 0),
                    stop=(si_ == n_st - 1),
                    perf_mode=PM.DoubleRow,
                )
        _lbl(f"MM{i}.deq")
        # dequant: out = ps * deq (bias came in through the matmul).
        # high_priority: psum release must never wait behind hoisted
        # quant work, or the 2-buffer psum ring stalls the PE.
        o_t = outp.tile([P, D_OUT], F16, tag="o")
        A = _env("K_DQA", 2048)
        if A > 0:
            nc.scalar.activation(
                o_t[:, :A], ps[:, :A], AF.Identity, scale=deq[:, 0:1]
            )
        if A < D_OUT:
            nc.vector.tensor_scalar(
                o_t[:, A:], ps[:, A:], deq[:, 0:1], None, OP.mult
            )
        return o_t

    def store_stage(i, o_t):
        _lbl(f"ST{i}")
        r = slice(i * P, (i + 1) * P)
        nc.sync.dma_start(o_d[r, :D_OUT // 2], o_t[:, :D_OUT // 2])
        nc.sync.dma_start(o_d[r, D_OUT // 2 :], o_t[:, D_OUT // 2 :])

    # 4-deep software pipeline; every SP dma's inputs complete >=1 step early
    lagT = _env("K_LAGT", 2)   # quant -> transpose
    lagM = _env("K_LAGM", 1)   # transpose -> matmul
    lagS = _env("K_LAGS", 1)   # dequant -> store
    sQ, sT, sM = {}, {}, {}
    for step in range(n_tiles + lagT + lagM + lagS):
        if step < n_tiles:
            sQ[step] = quant_stage(step)
        j = step - lagT
        if 0 <= j < n_tiles:
            sT[j] = transpose_stage(j, sQ.pop(j))
        j = step - lagT - lagM
        if 0 <= j < n_tiles:
            sM[j] = mm_stage(j, sT.pop(j))
        j = step - lagT - lagM - lagS
        if 0 <= j < n_tiles:
            store_stage(j, sM.pop(j))


def prep_inputs(x, quantized_weight, scale, bias):
    x = np.asarray(x, dtype=np.float32)
    quantized_weight = np.asarray(quantized_weight, dtype=np.float32)
    scale = np.asarray(scale, dtype=np.float32)
    bias = np.asarray(bias, dtype=np.float32)
    f8 = ml_dtypes.float8_e4m3fn
    xf = np.ascontiguousarray(x.reshape(-1, D_IN)).astype(np.float16)
    wT = quantized_weight.T.astype(np.float32)  # [d, o]
    # exact half: we[p, cc, o] = wT[128cc + p, o]
    we = wT[:ED].reshape(NEB, P, D_OUT).transpose(1, 0, 2).reshape(P, NEB * D_OUT)
    # plain half: wp[p, j, b, s, o] = wT[1024 + 512j + 256b + 2p + s, o]
    wp = (
        wT[ED:]
        .reshape(NPB // 2, 2, P, 2, D_OUT)    # [j, b, p, s, o]
        .transpose(2, 0, 1, 3, 4)
        .reshape(P, NPB * 2 * D_OUT)
    )
    w_prep = np.ascontiguousarray(np.concatenate([we, wp], axis=1)).astype(f8)
    wbias = np.ascontiguousarray((16.0 * bias).reshape(1, D_OUT)).astype(f8)
    rs16_bc = np.full((P, 1), np.float32(1.0 / (16.0 * scale)), dtype=np.float32)
    scale_bc = np.full((P, 1), np.float32(scale), dtype=np.float32)
    return xf, w_prep, wbias, rs16_bc, scale_bc


_NC_CACHE = {}


def get_nc(n_tiles=N_TILES):
    if n_tiles not in _NC_CACHE:
        _NC_CACHE[n_tiles] = build_kernel(n_tiles)
    return _NC_CACHE[n_tiles]


def kernel(x, quantized_weight, scale, bias, _trace=False):
    xf, w_prep, wbias, rs16_bc, scale_bc = prep_inputs(x, quantized_weight, scale, bias)
    in_maps = [
        {
            "x": xf[i * TPC : (i + 1) * TPC],
            "w": w_prep,
            "wbias": wbias,
            "rs16": rs16_bc,
            "scale": scale_bc,
        }
        for i in range(N_CORES)
    ]
    nc = get_nc()
    res = run_bass_kernel_spmd(nc, in_maps, list(range(N_CORES)), trace=_trace)
    out = np.concatenate([res.results[i]["out"] for i in range(N_CORES)], axis=0)
    out = out.reshape(B, S, D_OUT).astype(np.float32)
    if _trace:
        return out, res
    return out
